# revision 7
# baseline (speedup 1.0000x reference)
"""Trainium2 Bass kernel for AttLayer-style attention pooling.

Computes, for x[B, T, D], W[D, A], b[A], u[A, 1]:
    uit = tanh(x @ W + b)            # [B, T, A]
    z   = uit @ u[:, 0]              # [B, T]
    e   = exp(z)
    a   = e / (sum_t e + 1e-7)
    y   = einsum('btd,bt->bd', x, a) # [B, D]

Sharding: pure data parallel over batch. Each of the 8 NeuronCores gets
B/8 = 8 batches; W/b/u are replicated; no cross-core communication.

Per-core dataflow (all matmuls in bf16 with f32 PSUM accumulation):
  1. SWDGE cast-DMA loads one batch of x as bf16 in a [128, 16, 256]
     tile, partition p holding rows t = p*16 + i (16 KiB contiguous HBM
     reads per partition).
  2. DMA xbar transposes build xT tiles [d, i, p] for the first matmul
     (PE contracts the partition axis, so D must sit on partitions).
  3. mm1: W-chunk-stationary matmuls produce uitT [A, t'] in PSUM;
     ScalarE applies tanh(+b) into SBUF as bf16.
  4. mm2: uitT 128-column chunks as stationary against u -> z in PSUM
     [p, i]; ScalarE exp with accum_out gives e and per-partition sums.
  5. mm3: e columns as stationary weights against natural x tiles
     accumulate the weighted sum y' [1, D]; a ones-matmul folds the
     per-partition sums into the softmax denominator.
  6. VectorE normalizes y'/(S+eps); result DMAs out.
"""

from contextlib import ExitStack

import numpy as np

import concourse.bass as bass
import concourse.tile as tile
from concourse import mybir
from concourse.bass_utils import run_bass_kernel_spmd

N_CORES = 8
B, T, D, A = 64, 2048, 256, 128
BC = B // N_CORES  # batches per core
I = T // 128  # 16 inner t-blocks; partition p holds t = p*I + i
EPS = 1e-7

F32 = mybir.dt.float32
BF16 = mybir.dt.bfloat16
TANH = mybir.ActivationFunctionType.Tanh
EXP = mybir.ActivationFunctionType.Exp


def _split_multi_waits(nc):
    """Hoist all-but-one sem wait off every instruction onto no-ops.

    The walrus build in this container rejects instructions carrying
    more than one sync-wait command (CoreV3 setupSyncWait). A no-op on
    the same engine immediately before the instruction is semantically
    identical: the engine blocks on each wait in sequence.
    """
    counter = [0]

    def fresh_nop(engine, wait):
        counter[0] += 1
        n = mybir.InstNoOp(name=f"I-waitsplit-{counter[0]}", ins=[], outs=[])
        n.engine = engine
        n.sync_info = mybir.SyncInfo(on_wait=[wait], on_update=[])
        return n

    for fn in nc.m.functions:
        for blk in fn.blocks:
            changed = False
            out = []
            for inst in blk.instructions:
                si = inst.sync_info
                if si is not None and si.on_wait and len(si.on_wait) > 1:
                    waits = list(si.on_wait)
                    for w in waits[:-1]:
                        out.append(fresh_nop(inst.engine, w))
                    si.on_wait = waits[-1:]
                    changed = True
                out.append(inst)
            if changed:
                blk.instructions = out


def _emit_body(ctx, tc, x, W, b, u, out):
    nc = tc.nc

    singles = ctx.enter_context(tc.tile_pool(name="singles", bufs=1))
    xpool = ctx.enter_context(tc.tile_pool(name="xnat", bufs=3))
    xtpool = ctx.enter_context(tc.tile_pool(name="xt", bufs=2))
    upool = ctx.enter_context(tc.tile_pool(name="uit", bufs=2))
    spool = ctx.enter_context(tc.tile_pool(name="small", bufs=3))
    pu_pool = ctx.enter_context(tc.tile_pool(name="pu", bufs=3, space="PSUM"))
    pa_pool = ctx.enter_context(tc.tile_pool(name="pa", bufs=2, space="PSUM"))
    py_pool = ctx.enter_context(tc.tile_pool(name="py", bufs=2, space="PSUM"))

    # Replicated parameters. W is consumed as two [128, A] K-chunks.
    W_f = singles.tile([128, 2, A], F32)
    nc.gpsimd.dma_start(W_f[:], W.ap().rearrange("(c k) a -> k c a", c=2))
    W_bf = singles.tile([128, 2, A], BF16)
    nc.vector.tensor_copy(W_bf[:], W_f[:])
    b_sb = singles.tile([A, 1], F32)
    nc.gpsimd.dma_start(b_sb[:], b.ap().rearrange("(a o) -> a o", o=1))
    u_f = singles.tile([A, 1], F32)
    nc.gpsimd.dma_start(u_f[:], u.ap())
    u_bf = singles.tile([A, 1], BF16)
    nc.vector.tensor_copy(u_bf[:], u_f[:])
    ones_f = singles.tile([128, 1], F32)
    nc.vector.memset(ones_f[:], 1.0)

    for bi in range(BC):
        # Natural-layout x for this batch, cast to bf16 during the DMA.
        x_nat = xpool.tile([128, I, D], BF16, tag="xnat")
        nc.gpsimd.dma_start(
            x_nat[:], x.ap()[bi].rearrange("(p i) d -> p i d", i=I)
        )

        # xbar-transposed copies: xt{0,1}[d, i, p] for d-chunks 0/1.
        xt0 = xtpool.tile([128, I, 128], BF16, tag="xt0")
        xt1 = xtpool.tile([128, I, 128], BF16, tag="xt1")
        for i in range(I):
            nc.sync.dma_start(xt0[:, i, :], x_nat[:, i, 0:128], transpose=True)
            nc.sync.dma_start(xt1[:, i, :], x_nat[:, i, 128:256], transpose=True)

        # mm1 + tanh: uitT[a, i, p] = tanh(sum_d W[d,a] x[t,d] + b[a])
        uitT = upool.tile([A, I, 128], BF16, tag="uitT")
        for g in range(I // 4):
            pug = pu_pool.tile([A, 512], F32, tag="pu")
            for kc, xt in enumerate((xt0, xt1)):
                nc.tensor.matmul(
                    pug[:],
                    W_bf[:, kc, :],
                    xt[:, 4 * g : 4 * g + 4, :],
                    start=(kc == 0),
                    stop=(kc == 1),
                )
            nc.scalar.activation(
                uitT[:, 4 * g : 4 * g + 4, :], pug[:], TANH, bias=b_sb[:]
            )

        # mm2: z[p, i] = sum_a uitT[a, i, p] * u[a]
        pait = pa_pool.tile([128, I], F32, tag="pa")
        for i in range(I):
            nc.tensor.matmul(
                pait[:, i : i + 1], uitT[:, i, :], u_bf[:], start=True, stop=True
            )

        # exp with fused per-partition row sums.
        e_f = spool.tile([128, I], F32, tag="ef")
        s1 = spool.tile([128, 1], F32, tag="s1")
        nc.scalar.activation(e_f[:], pait[:], EXP, accum_out=s1[:])
        e_bf = spool.tile([128, I], BF16, tag="ebf")
        nc.vector.tensor_copy(e_bf[:], e_f[:])

        # mm3: y'[d] = sum_t e[t] x[t, d]; plus S = sum_p s1[p].
        pys = py_pool.tile([1, 512], F32, tag="py")
        for i in range(I):
            nc.tensor.matmul(
                pys[:, 0:D],
                e_bf[:, i : i + 1],
                x_nat[:, i, :],
                start=(i == 0),
                stop=(i == I - 1),
            )
        nc.tensor.matmul(pys[:, D : D + 1], s1[:], ones_f[:], start=True, stop=True)

        # y = y' / (S + eps)
        s_sb = spool.tile([1, 1], F32, tag="ssb")
        nc.vector.tensor_scalar_add(s_sb[:], pys[:, D : D + 1], EPS)
        r_sb = spool.tile([1, 1], F32, tag="rsb")
        nc.vector.reciprocal(r_sb[:], s_sb[:])
        y_sb = spool.tile([1, D], F32, tag="ysb")
        nc.vector.tensor_scalar_mul(y_sb[:], pys[:, 0:D], r_sb[:])
        nc.sync.dma_start(out.ap()[bi : bi + 1, :], y_sb[:])


_NC_CACHE = None


def _build_nc():
    global _NC_CACHE
    if _NC_CACHE is not None:
        return _NC_CACHE
    nc = bass.Bass()
    x = nc.declare_dram_parameter("x", [BC, T, D], F32, isOutput=False)
    W = nc.declare_dram_parameter("W", [D, A], F32, isOutput=False)
    b = nc.declare_dram_parameter("b", [A], F32, isOutput=False)
    u = nc.declare_dram_parameter("u", [A, 1], F32, isOutput=False)
    out = nc.declare_dram_parameter("out", [BC, D], F32, isOutput=True)
    with tile.TileContext(nc) as tc, ExitStack() as ctx:
        _emit_body(ctx, tc, x, W, b, u, out)
    _split_multi_waits(nc)
    _NC_CACHE = nc
    return nc


def make_in_maps(x, W, b, u):
    x = np.ascontiguousarray(x, dtype=np.float32)
    W = np.ascontiguousarray(W, dtype=np.float32)
    b = np.ascontiguousarray(b, dtype=np.float32)
    u = np.ascontiguousarray(u, dtype=np.float32)
    return [
        {"x": x[c * BC : (c + 1) * BC], "W": W, "b": b, "u": u}
        for c in range(N_CORES)
    ]


def kernel(x, W, b, u):
    nc = _build_nc()
    res = run_bass_kernel_spmd(nc, make_in_maps(x, W, b, u), list(range(N_CORES)))
    return np.concatenate([r["out"] for r in res.results], axis=0)


# revision 14
# speedup vs baseline: 1.5348x; 1.5348x over previous
"""Trainium2 Bass kernel for AttLayer-style attention pooling.

Computes, for x[B, T, D], W[D, A], b[A], u[A, 1]:
    uit = tanh(x @ W + b)            # [B, T, A]
    z   = uit @ u[:, 0]              # [B, T]
    e   = exp(z)
    a   = e / (sum_t e + 1e-7)
    y   = einsum('btd,bt->bd', x, a) # [B, D]

Sharding: pure data parallel over batch. Each of the 8 NeuronCores gets
B/8 = 8 batches; W/b/u are replicated; no cross-core communication.

Per-core dataflow (all matmuls in bf16 with f32 PSUM accumulation):
  1. SWDGE cast-DMA loads one batch of x as bf16 in a [128, 16, 256]
     tile, partition p holding rows t = p*16 + i (16 KiB contiguous HBM
     reads per partition).
  2. DMA xbar transposes build xT tiles [d, i, p] for the first matmul
     (PE contracts the partition axis, so D must sit on partitions).
  3. mm1: W-chunk-stationary matmuls produce uitT [A, t'] in PSUM;
     ScalarE applies tanh(+b) into SBUF as bf16.
  4. mm2: uitT 128-column chunks as stationary against u -> z in PSUM
     [p, i]; ScalarE exp with accum_out gives e and per-partition sums.
  5. mm3: e columns as stationary weights against natural x tiles
     accumulate the weighted sum y' [1, D]; a ones-matmul folds the
     per-partition sums into the softmax denominator.
  6. VectorE normalizes y'/(S+eps); result DMAs out.
"""

from contextlib import ExitStack

import numpy as np

import concourse.bass as bass
import concourse.tile as tile
from concourse import mybir
from concourse.bass_utils import run_bass_kernel_spmd
from concourse.masks import make_identity

N_CORES = 8
B, T, D, A = 64, 2048, 256, 128
BC = B // N_CORES  # batches per core
I = T // 128  # 16 inner t-blocks; partition p holds t = p*I + i
EPS = 1e-7

F32 = mybir.dt.float32
BF16 = mybir.dt.bfloat16
TANH = mybir.ActivationFunctionType.Tanh
EXP = mybir.ActivationFunctionType.Exp


# Instruction types whose CoreV3 ISA struct only has room for a single
# sync-wait command in this walrus build. Multi-wait instructions of
# these types get their extra waits hoisted onto preceding no-ops.
_SINGLE_WAIT_TYPES = {
    "InstDrain",
    "InstDmaTransposeAnt",
    "InstNoOp",
    "InstEventSemaphore",
}
_SPLIT_ALL = True


def _split_multi_waits(nc):
    """Hoist all-but-one sem wait off restricted instructions onto no-ops.

    The walrus build in this container rejects some instruction types
    carrying more than one sync-wait command (CoreV3 setupSyncWait). A
    no-op on the same engine immediately before the instruction is
    semantically identical: the engine blocks on each wait in sequence.
    """
    counter = [0]

    def fresh_nop(engine, wait):
        counter[0] += 1
        n = mybir.InstNoOp(name=f"I-waitsplit-{counter[0]}", ins=[], outs=[])
        n.engine = engine
        n.sync_info = mybir.SyncInfo(on_wait=[wait], on_update=[])
        nc.register_instruction(n)
        return n

    for fn in nc.m.functions:
        for blk in fn.blocks:
            changed = False
            out = []
            for inst in blk.instructions:
                si = inst.sync_info
                if (
                    si is not None
                    and si.on_wait
                    and len(si.on_wait) > 1
                    and (_SPLIT_ALL or type(inst).__name__ in _SINGLE_WAIT_TYPES)
                ):
                    waits = list(si.on_wait)
                    for w in waits[:-1]:
                        out.append(fresh_nop(inst.engine, w))
                    si.on_wait = waits[-1:]
                    changed = True
                out.append(inst)
            if changed:
                blk.instructions = out


TRANSPOSE_MODE = "pe"  # "pe" (TensorE transpose + copy) or "xbar" (DMA)


def _emit_body(ctx, tc, x, W, b, u, out):
    nc = tc.nc

    singles = ctx.enter_context(tc.tile_pool(name="singles", bufs=1))
    xpool = ctx.enter_context(tc.tile_pool(name="xnat", bufs=3))
    xtpool = ctx.enter_context(tc.tile_pool(name="xt", bufs=2))
    upool = ctx.enter_context(tc.tile_pool(name="uit", bufs=2))
    spool = ctx.enter_context(tc.tile_pool(name="small", bufs=3))
    pu_pool = ctx.enter_context(tc.tile_pool(name="pu", bufs=2, space="PSUM"))
    pa_pool = ctx.enter_context(tc.tile_pool(name="pa", bufs=2, space="PSUM"))
    py_pool = ctx.enter_context(tc.tile_pool(name="py", bufs=2, space="PSUM"))
    if TRANSPOSE_MODE == "pe":
        tr_pool = ctx.enter_context(tc.tile_pool(name="tr", bufs=2, space="PSUM"))

    # Replicated parameters. W is consumed as two [128, A] K-chunks.
    W_f = singles.tile([128, 2, A], F32)
    nc.gpsimd.dma_start(W_f[:], W.ap().rearrange("(c k) a -> k c a", c=2))
    W_bf = singles.tile([128, 2, A], BF16)
    nc.vector.tensor_copy(W_bf[:], W_f[:])
    b_sb = singles.tile([A, 1], F32)
    nc.gpsimd.dma_start(b_sb[:], b.ap().rearrange("(a o) -> a o", o=1))
    u_f = singles.tile([A, 1], F32)
    nc.gpsimd.dma_start(u_f[:], u.ap())
    u_bf = singles.tile([A, 1], BF16)
    nc.vector.tensor_copy(u_bf[:], u_f[:])
    ones_f = singles.tile([128, 1], F32)
    nc.vector.memset(ones_f[:], 1.0)
    if TRANSPOSE_MODE == "pe":
        identity = singles.tile([128, 128], BF16)
        make_identity(nc, identity[:])

    for bi in range(BC):
        # Natural-layout x for this batch, cast to bf16 during the DMA.
        x_nat = xpool.tile([128, I, D], BF16, tag="xnat")
        nc.gpsimd.dma_start(
            x_nat[:], x.ap()[bi].rearrange("(p i) d -> p i d", i=I)
        )

        # Transposed copies: xt{0,1}[d, i, p] for d-chunks 0/1.
        xt0 = xtpool.tile([128, I, 128], BF16, tag="xt0")
        xt1 = xtpool.tile([128, I, 128], BF16, tag="xt1")
        if TRANSPOSE_MODE == "xbar":
            for i in range(I):
                nc.sync.dma_start(xt0[:, i, :], x_nat[:, i, 0:128], transpose=True)
                nc.sync.dma_start(xt1[:, i, :], x_nat[:, i, 128:256], transpose=True)
        else:
            # TensorE transpose: 8 [128,128] bf16 tiles per PSUM bank,
            # then one bulk PSUM->SBUF copy per bank (ACT/DVE alternate).
            for dc, xt in enumerate((xt0, xt1)):
                for g in range(I // 8):
                    pt = tr_pool.tile([128, 8, 128], BF16, tag="tr")
                    for ii in range(8):
                        nc.tensor.transpose(
                            pt[:, ii, :],
                            x_nat[:, 8 * g + ii, 128 * dc : 128 * (dc + 1)],
                            identity[:],
                        )
                    if dc == 0:
                        nc.scalar.copy(xt[:, 8 * g : 8 * g + 8, :], pt[:])
                    else:
                        nc.vector.tensor_copy(xt[:, 8 * g : 8 * g + 8, :], pt[:])

        # mm1 + tanh: uitT[a, i, p] = tanh(sum_d W[d,a] x[t,d] + b[a])
        uitT = upool.tile([A, I, 128], BF16, tag="uitT")
        for g in range(I // 4):
            pug = pu_pool.tile([A, 512], F32, tag="pu")
            for kc, xt in enumerate((xt0, xt1)):
                nc.tensor.matmul(
                    pug[:],
                    W_bf[:, kc, :],
                    xt[:, 4 * g : 4 * g + 4, :],
                    start=(kc == 0),
                    stop=(kc == 1),
                )
            nc.scalar.activation(
                uitT[:, 4 * g : 4 * g + 4, :], pug[:], TANH, bias=b_sb[:]
            )

        # mm2: z[p, i] = sum_a uitT[a, i, p] * u[a]
        pait = pa_pool.tile([128, I], F32, tag="pa")
        for i in range(I):
            nc.tensor.matmul(
                pait[:, i : i + 1], uitT[:, i, :], u_bf[:], start=True, stop=True
            )

        # exp with fused per-partition row sums.
        e_f = spool.tile([128, I], F32, tag="ef")
        s1 = spool.tile([128, 1], F32, tag="s1")
        nc.scalar.activation(e_f[:], pait[:], EXP, accum_out=s1[:])
        e_bf = spool.tile([128, I], BF16, tag="ebf")
        nc.vector.tensor_copy(e_bf[:], e_f[:])

        # mm3: y'[d] = sum_t e[t] x[t, d]; plus S = sum_p s1[p].
        pys = py_pool.tile([1, 512], F32, tag="py")
        for i in range(I):
            nc.tensor.matmul(
                pys[:, 0:D],
                e_bf[:, i : i + 1],
                x_nat[:, i, :],
                start=(i == 0),
                stop=(i == I - 1),
            )
        nc.tensor.matmul(pys[:, D : D + 1], s1[:], ones_f[:], start=True, stop=True)

        # y = y' / (S + eps)
        s_sb = spool.tile([1, 1], F32, tag="ssb")
        nc.vector.tensor_scalar_add(s_sb[:], pys[:, D : D + 1], EPS)
        r_sb = spool.tile([1, 1], F32, tag="rsb")
        nc.vector.reciprocal(r_sb[:], s_sb[:])
        y_sb = spool.tile([1, D], F32, tag="ysb")
        nc.vector.tensor_scalar_mul(y_sb[:], pys[:, 0:D], r_sb[:])
        nc.sync.dma_start(out.ap()[bi : bi + 1, :], y_sb[:])


_NC_CACHE = None


def _build_nc():
    global _NC_CACHE
    if _NC_CACHE is not None:
        return _NC_CACHE
    nc = bass.Bass()
    x = nc.declare_dram_parameter("x", [BC, T, D], F32, isOutput=False)
    W = nc.declare_dram_parameter("W", [D, A], F32, isOutput=False)
    b = nc.declare_dram_parameter("b", [A], F32, isOutput=False)
    u = nc.declare_dram_parameter("u", [A, 1], F32, isOutput=False)
    out = nc.declare_dram_parameter("out", [BC, D], F32, isOutput=True)
    with tile.TileContext(nc) as tc, ExitStack() as ctx:
        _emit_body(ctx, tc, x, W, b, u, out)
    _split_multi_waits(nc)
    _NC_CACHE = nc
    return nc


def make_in_maps(x, W, b, u):
    x = np.ascontiguousarray(x, dtype=np.float32)
    W = np.ascontiguousarray(W, dtype=np.float32)
    b = np.ascontiguousarray(b, dtype=np.float32)
    u = np.ascontiguousarray(u, dtype=np.float32)
    return [
        {"x": x[c * BC : (c + 1) * BC], "W": W, "b": b, "u": u}
        for c in range(N_CORES)
    ]


def kernel(x, W, b, u):
    nc = _build_nc()
    res = run_bass_kernel_spmd(nc, make_in_maps(x, W, b, u), list(range(N_CORES)))
    return np.concatenate([r["out"] for r in res.results], axis=0)


# revision 20
# speedup vs baseline: 17.8208x; 11.6115x over previous
"""Trainium2 Bass kernel for AttLayer-style attention pooling.

Computes, for x[B, T, D], W[D, A], b[A], u[A, 1]:
    uit = tanh(x @ W + b)            # [B, T, A]
    z   = uit @ u[:, 0]              # [B, T]
    e   = exp(z)
    a   = e / (sum_t e + 1e-7)
    y   = einsum('btd,bt->bd', x, a) # [B, D]

Sharding: pure data parallel over batch. Each of the 8 NeuronCores gets
B/8 = 8 batches; W/b/u are replicated; no cross-core communication.

Per-core dataflow (all matmuls in bf16 with f32 PSUM accumulation):
  1. SWDGE cast-DMA loads one batch of x as bf16 in a [128, 16, 256]
     tile, partition p holding rows t = p*16 + i (16 KiB contiguous HBM
     reads per partition).
  2. DMA xbar transposes build xT tiles [d, i, p] for the first matmul
     (PE contracts the partition axis, so D must sit on partitions).
  3. mm1: W-chunk-stationary matmuls produce uitT [A, t'] in PSUM;
     ScalarE applies tanh(+b) into SBUF as bf16.
  4. mm2: uitT 128-column chunks as stationary against u -> z in PSUM
     [p, i]; ScalarE exp with accum_out gives e and per-partition sums.
  5. mm3: e columns as stationary weights against natural x tiles
     accumulate the weighted sum y' [1, D]; a ones-matmul folds the
     per-partition sums into the softmax denominator.
  6. VectorE normalizes y'/(S+eps); result DMAs out.
"""

from contextlib import ExitStack

import numpy as np

import concourse.bass as bass
import concourse.tile as tile
from concourse import mybir
from concourse.bass_utils import run_bass_kernel_spmd
from concourse.masks import make_identity

N_CORES = 8
B, T, D, A = 64, 2048, 256, 128
BC = B // N_CORES  # batches per core
I = T // 128  # 16 inner t-blocks; partition p holds t = p*I + i
EPS = 1e-7

F32 = mybir.dt.float32
BF16 = mybir.dt.bfloat16
TANH = mybir.ActivationFunctionType.Tanh
EXP = mybir.ActivationFunctionType.Exp


# Instruction types whose CoreV3 ISA struct only has room for a single
# sync-wait command in this walrus build. Multi-wait instructions of
# these types get their extra waits hoisted onto preceding no-ops.
_SINGLE_WAIT_TYPES = {
    "InstDrain",
    "InstDmaTransposeAnt",
    "InstNoOp",
    "InstEventSemaphore",
}
_SPLIT_ALL = True


def _split_multi_waits(nc):
    """Hoist all-but-one sem wait off restricted instructions onto no-ops.

    The walrus build in this container rejects some instruction types
    carrying more than one sync-wait command (CoreV3 setupSyncWait). A
    no-op on the same engine immediately before the instruction is
    semantically identical: the engine blocks on each wait in sequence.
    """
    counter = [0]

    def fresh_nop(engine, wait):
        counter[0] += 1
        n = mybir.InstNoOp(name=f"I-waitsplit-{counter[0]}", ins=[], outs=[])
        n.engine = engine
        n.sync_info = mybir.SyncInfo(on_wait=[wait], on_update=[])
        nc.register_instruction(n)
        return n

    for fn in nc.m.functions:
        for blk in fn.blocks:
            changed = False
            out = []
            for inst in blk.instructions:
                si = inst.sync_info
                if (
                    si is not None
                    and si.on_wait
                    and len(si.on_wait) > 1
                    and (_SPLIT_ALL or type(inst).__name__ in _SINGLE_WAIT_TYPES)
                ):
                    waits = list(si.on_wait)
                    for w in waits[:-1]:
                        out.append(fresh_nop(inst.engine, w))
                    si.on_wait = waits[-1:]
                    changed = True
                out.append(inst)
            if changed:
                blk.instructions = out


TRANSPOSE_MODE = "pe"  # "pe" (TensorE transpose + copy) or "xbar" (DMA)
COPY_SPLIT = "dve"  # "dc": ACT gets d-chunk 0, DVE chunk 1; "dve"/"act": all one engine


def _emit_body(ctx, tc, x, W, b, u, out, repeat=1):
    nc = tc.nc

    singles = ctx.enter_context(tc.tile_pool(name="singles", bufs=1))
    xpool = ctx.enter_context(tc.tile_pool(name="xnat", bufs=3))
    xtpool = ctx.enter_context(tc.tile_pool(name="xt", bufs=2))
    upool = ctx.enter_context(tc.tile_pool(name="uit", bufs=2))
    spool = ctx.enter_context(tc.tile_pool(name="small", bufs=3))
    pu_pool = ctx.enter_context(tc.tile_pool(name="pu", bufs=2, space="PSUM"))
    pa_pool = ctx.enter_context(tc.tile_pool(name="pa", bufs=2, space="PSUM"))
    py_pool = ctx.enter_context(tc.tile_pool(name="py", bufs=2, space="PSUM"))
    if TRANSPOSE_MODE == "pe":
        tr_pool = ctx.enter_context(tc.tile_pool(name="tr", bufs=2, space="PSUM"))

    # Replicated parameters. W is consumed as two [128, A] K-chunks.
    W_f = singles.tile([128, 2, A], F32)
    nc.gpsimd.dma_start(W_f[:], W.ap().rearrange("(c k) a -> k c a", c=2))
    W_bf = singles.tile([128, 2, A], BF16)
    nc.vector.tensor_copy(W_bf[:], W_f[:])
    b_sb = singles.tile([A, 1], F32)
    nc.gpsimd.dma_start(b_sb[:], b.ap().rearrange("(a o) -> a o", o=1))
    u_f = singles.tile([A, 1], F32)
    nc.gpsimd.dma_start(u_f[:], u.ap())
    u_bf = singles.tile([A, 1], BF16)
    nc.vector.tensor_copy(u_bf[:], u_f[:])
    ones_f = singles.tile([128, 1], F32)
    nc.vector.memset(ones_f[:], 1.0)
    if TRANSPOSE_MODE == "pe":
        identity = singles.tile([128, 128], BF16)
        make_identity(nc, identity[:])

    for bi in [i for _ in range(repeat) for i in range(BC)]:
        # Natural-layout x for this batch, cast to bf16 during the DMA.
        x_nat = xpool.tile([128, I, D], BF16, tag="xnat")
        nc.gpsimd.dma_start(
            x_nat[:], x.ap()[bi].rearrange("(p i) d -> p i d", i=I)
        )

        # Transposed copies: xt{0,1}[d, i, p] for d-chunks 0/1.
        xt0 = xtpool.tile([128, I, 128], BF16, tag="xt0")
        xt1 = xtpool.tile([128, I, 128], BF16, tag="xt1")
        if TRANSPOSE_MODE == "xbar":
            for i in range(I):
                nc.sync.dma_start(xt0[:, i, :], x_nat[:, i, 0:128], transpose=True)
                nc.sync.dma_start(xt1[:, i, :], x_nat[:, i, 128:256], transpose=True)
        else:
            # TensorE transpose: 8 [128,128] bf16 tiles per PSUM bank,
            # then one bulk PSUM->SBUF copy per bank (ACT/DVE alternate).
            for dc, xt in enumerate((xt0, xt1)):
                for g in range(I // 8):
                    pt = tr_pool.tile([128, 8, 128], BF16, tag="tr")
                    for ii in range(8):
                        nc.tensor.transpose(
                            pt[:, ii, :],
                            x_nat[:, 8 * g + ii, 128 * dc : 128 * (dc + 1)],
                            identity[:],
                        )
                    on_act = {"dc": dc == 0, "act": True, "dve": False}[COPY_SPLIT]
                    if on_act:
                        nc.scalar.copy(xt[:, 8 * g : 8 * g + 8, :], pt[:])
                    else:
                        nc.vector.tensor_copy(xt[:, 8 * g : 8 * g + 8, :], pt[:])

        # mm1 + tanh: uitT[a, i, p] = tanh(sum_d W[d,a] x[t,d] + b[a])
        uitT = upool.tile([A, I, 128], BF16, tag="uitT")
        for g in range(I // 4):
            pug = pu_pool.tile([A, 512], F32, tag="pu")
            for kc, xt in enumerate((xt0, xt1)):
                nc.tensor.matmul(
                    pug[:],
                    W_bf[:, kc, :],
                    xt[:, 4 * g : 4 * g + 4, :],
                    start=(kc == 0),
                    stop=(kc == 1),
                )
            nc.scalar.activation(
                uitT[:, 4 * g : 4 * g + 4, :], pug[:], TANH, bias=b_sb[:]
            )

        # mm2: z[p, i] = sum_a uitT[a, i, p] * u[a]
        pait = pa_pool.tile([128, I], F32, tag="pa")
        for i in range(I):
            nc.tensor.matmul(
                pait[:, i : i + 1], uitT[:, i, :], u_bf[:], start=True, stop=True
            )

        # exp with fused per-partition row sums.
        e_f = spool.tile([128, I], F32, tag="ef")
        s1 = spool.tile([128, 1], F32, tag="s1")
        nc.scalar.activation(e_f[:], pait[:], EXP, accum_out=s1[:])
        e_bf = spool.tile([128, I], BF16, tag="ebf")
        nc.vector.tensor_copy(e_bf[:], e_f[:])

        # mm3: y'[d] = sum_t e[t] x[t, d]; plus S = sum_p s1[p].
        pys = py_pool.tile([1, 512], F32, tag="py")
        for i in range(I):
            nc.tensor.matmul(
                pys[:, 0:D],
                e_bf[:, i : i + 1],
                x_nat[:, i, :],
                start=(i == 0),
                stop=(i == I - 1),
            )
        nc.tensor.matmul(pys[:, D : D + 1], s1[:], ones_f[:], start=True, stop=True)

        # y = y' / (S + eps)
        s_sb = spool.tile([1, 1], F32, tag="ssb")
        nc.vector.tensor_scalar_add(s_sb[:], pys[:, D : D + 1], EPS)
        r_sb = spool.tile([1, 1], F32, tag="rsb")
        nc.vector.reciprocal(r_sb[:], s_sb[:])
        y_sb = spool.tile([1, D], F32, tag="ysb")
        nc.vector.tensor_scalar_mul(y_sb[:], pys[:, 0:D], r_sb[:])
        nc.sync.dma_start(out.ap()[bi : bi + 1, :], y_sb[:])


_NC_CACHE = {}


def _build_nc(repeat=1):
    if repeat in _NC_CACHE:
        return _NC_CACHE[repeat]
    nc = bass.Bass()
    x = nc.declare_dram_parameter("x", [BC, T, D], F32, isOutput=False)
    W = nc.declare_dram_parameter("W", [D, A], F32, isOutput=False)
    b = nc.declare_dram_parameter("b", [A], F32, isOutput=False)
    u = nc.declare_dram_parameter("u", [A, 1], F32, isOutput=False)
    out = nc.declare_dram_parameter("out", [BC, D], F32, isOutput=True)
    with tile.TileContext(nc) as tc, ExitStack() as ctx:
        _emit_body(ctx, tc, x, W, b, u, out, repeat=repeat)
    _split_multi_waits(nc)
    _NC_CACHE[repeat] = nc
    return nc


def make_in_maps(x, W, b, u):
    x = np.ascontiguousarray(x, dtype=np.float32)
    W = np.ascontiguousarray(W, dtype=np.float32)
    b = np.ascontiguousarray(b, dtype=np.float32)
    u = np.ascontiguousarray(u, dtype=np.float32)
    return [
        {"x": x[c * BC : (c + 1) * BC], "W": W, "b": b, "u": u}
        for c in range(N_CORES)
    ]


def kernel(x, W, b, u):
    nc = _build_nc()
    res = run_bass_kernel_spmd(nc, make_in_maps(x, W, b, u), list(range(N_CORES)))
    return np.concatenate([r["out"] for r in res.results], axis=0)
